# revision 1
# baseline (speedup 1.0000x reference)
"""Additive-attention (ContentAttender) Bass kernel for 8 TRN2 NeuronCores.

Problem: B=4, NQ=512, NK=512, D=128, H=32
  kh = keys @ Wk; qh = queries @ Wq
  logits[b,q,k] = w2 . tanh(qh[b,q] + kh[b,k] + b1) + b2
  out = softmax_k(logits) @ keys

Sharding: data-parallel over (batch x query-half) -> 8 cores, each core
handles one batch's 256 queries vs all 512 keys. No collectives.

Per-core pipeline: queries packed 4-per-32-partition-group; the (q,k,h)
tanh tensor is built as 64 DVE broadcast-adds (khT4 + per-group query
bias, bf16 2x mode) feeding big-chunk ScalarE tanh (the roofline:
~4.2M tanh elems/core at 1 elem/cycle/lane), and the h-contraction with
w2 runs on the TensorEngine via a host-built block-diagonal weight
matrix into 32-row PSUM slices. Softmax skips max-subtraction
(|logits| <= sum|w2| ~ 3, safe in fp32); b2 dropped (softmax
shift-invariant). Normalization deferred: context = (exp @ keys) / rowsum.
Bank A's softmax exp runs in the mid-stream ACT gap; its transposes and
context matmuls are deferred past the final logits matmuls so the
terminal exp's TensorEngine dependencies are never queued behind them.
"""

import contextlib

import numpy as np
import ml_dtypes

import concourse.bass as bass  # noqa: F401
import concourse.mybir as mybir
import concourse.tile as tile
from concourse import bacc
from concourse.bass_utils import run_bass_kernel_spmd

F32 = mybir.dt.float32
BF16 = mybir.dt.bfloat16
AF = mybir.ActivationFunctionType

B, NQ, NK, D, H = 4, 512, 512, 128, 32
NQC = NQ // 2          # queries per core = 256
NG = NQC // 4          # query groups per core = 64

# bundleA columns: keysT | queriesT | Wk | Wq | b14
KT0, QT0, WK0, WQ0, B140 = 0, 512, 768, 800, 832
NCOLA = 833
# bundleB columns: kctx | W2D | identity
KX0, W2D0, ID0 = 0, 512, 768
NCOLB = 896

CHUNKS = [4, 8, 12, 16, 8, 8, 4, 3, 1]  # groups per tanh chunk (sum = 64)

_CACHED_NC = None


def _build_nc():
    nc = bacc.Bacc("TRN2", target_bir_lowering=False, debug=False)

    bundleA = nc.declare_dram_parameter("bundleA", [128, NCOLA], BF16, isOutput=False)
    bundleB = nc.declare_dram_parameter("bundleB", [128, NCOLB], BF16, isOutput=False)
    out = nc.declare_dram_parameter("out", [NQC, D], F32, isOutput=True)

    with tile.TileContext(nc) as tc, contextlib.ExitStack() as ctx:
        cpool = ctx.enter_context(tc.tile_pool(name="consts", bufs=1))
        spool = ctx.enter_context(tc.tile_pool(name="schunk", bufs=3))
        epool = ctx.enter_context(tc.tile_pool(name="softmax", bufs=2))
        ps_kh = ctx.enter_context(tc.tile_pool(name="ps_kh", bufs=1, space="PSUM"))
        ps_qb = ctx.enter_context(tc.tile_pool(name="ps_qb", bufs=1, space="PSUM"))
        ps_logits = ctx.enter_context(
            tc.tile_pool(name="ps_logits", bufs=2, space="PSUM")
        )
        ps_tr = ctx.enter_context(tc.tile_pool(name="ps_tr", bufs=2, space="PSUM"))
        ps_ctx = ctx.enter_context(tc.tile_pool(name="ps_ctx", bufs=2, space="PSUM"))

        bA = cpool.tile([128, NCOLA], BF16, tag="bA")
        nc.sync.dma_start(bA[:], bundleA[:])
        bB = cpool.tile([128, NCOLB], BF16, tag="bB")
        nc.sync.dma_start(bB[:], bundleB[:])

        kT = bA[:, KT0 : KT0 + NK]
        qT = bA[:, QT0 : QT0 + NQC]
        Wk_sb = bA[:, WK0 : WK0 + H]
        Wq_sb = bA[:, WQ0 : WQ0 + H]
        b14 = bA[:, B140 : B140 + 1]
        kctx_sb = bB[:, KX0 : KX0 + NK]
        W2D_sb = bB[:, W2D0 : W2D0 + 8 * H]
        id_sb = bB[:, ID0 : ID0 + 128]

        # khT4[(j,h), k] = (keys @ Wk)[k, h] replicated on 4 partition groups
        khT4_ps = ps_kh.tile([128, NK], F32, tag="khps", name="khT4_ps")
        for j in range(4):
            nc.tensor.matmul(
                khT4_ps[32 * j : 32 * j + 32, :],
                Wk_sb,
                kT,
                start=True,
                stop=True,
                tile_position=(0, 32 * j),
            )
        khT4 = cpool.tile([128, NK], BF16, tag="khT4")
        nc.vector.tensor_copy(khT4[:], khT4_ps[:])

        # QB4[(j,h), g] = qh[64j + g, h] + b1[h]   (b1 folded on copy-out)
        b14f = cpool.tile([128, 1], F32, tag="b14f")
        nc.vector.tensor_copy(b14f[:], b14)
        QB4_ps = ps_qb.tile([128, NG], F32, tag="qbps", name="QB4_ps")
        for j in range(4):
            nc.tensor.matmul(
                QB4_ps[32 * j : 32 * j + 32, :],
                Wq_sb,
                qT[:, NG * j : NG * (j + 1)],
                start=True,
                stop=True,
                tile_position=(0, 32 * j),
            )
        QB4 = cpool.tile([128, NG], F32, tag="QB4")
        nc.vector.tensor_scalar_add(QB4[:], QB4_ps[:], b14f[:])

        logits_ps = [None, None]
        g0 = 0

        def emit_chunk(n, bias_mode=False):
            nonlocal g0
            T = spool.tile([128, max(CHUNKS) * NK], BF16, tag="T", name="T")
            if bias_mode:
                # tanh straight off khT4 with the query bias in the ACT
                # affine stage: no DVE adds on the pipeline-ramp chunk.
                for gl in range(n):
                    g = g0 + gl
                    nc.scalar.activation(
                        T[:, NK * gl : NK * (gl + 1)],
                        khT4[:],
                        AF.Tanh,
                        bias=QB4[:, g : g + 1],
                    )
            else:
                S = spool.tile([128, max(CHUNKS) * NK], BF16, tag="S", name="S")
                for gl in range(n):
                    g = g0 + gl
                    nc.vector.tensor_scalar_add(
                        S[:, NK * gl : NK * (gl + 1)], khT4[:], QB4[:, g : g + 1]
                    )
                nc.scalar.activation(T[:, : NK * n], S[:, : NK * n], AF.Tanh)
            for gl in range(n):
                g = g0 + gl
                beta = g // 32
                s = (g // 8) % 4
                g8 = g % 8
                if logits_ps[beta] is None:
                    logits_ps[beta] = ps_logits.tile(
                        [128, NK], F32, tag="logits", name=f"logits{beta}"
                    )
                nc.tensor.matmul(
                    logits_ps[beta][32 * s : 32 * s + 32, :],
                    W2D_sb[:, 32 * g8 : 32 * g8 + 32],
                    T[:, NK * gl : NK * (gl + 1)],
                    start=(g8 == 0),
                    stop=(g8 == 7),
                    tile_position=(0, 32 * s),
                )
            g0 += n

        tails = {}

        def emit_tail_exp(beta):
            E = epool.tile([128, NK], BF16, tag="E", name="E")
            rs = epool.tile([128, 1], F32, tag="rs", name="rs")
            nc.scalar.activation(E[:], logits_ps[beta][:], AF.Exp, accum_out=rs[:])
            rr = epool.tile([128, 1], F32, tag="rr", name="rr")
            nc.vector.reciprocal(rr[:], rs[:])
            tails[beta] = (E, rr)

        def emit_tail_rest(beta):
            E, rr = tails[beta]
            ET = epool.tile([128, NK], BF16, tag="ET", name="ET")
            for t in range(4):
                trp = ps_tr.tile([128, 128], BF16, tag="tr", name="trp")
                nc.tensor.transpose(trp[:], E[:, 128 * t : 128 * (t + 1)], id_sb)
                nc.vector.tensor_copy(ET[:, 128 * t : 128 * (t + 1)], trp[:])
            ctxp = ps_ctx.tile([128, D], F32, tag="ctx", name="ctxp")
            for t in range(4):
                nc.tensor.matmul(
                    ctxp[:],
                    ET[:, 128 * t : 128 * (t + 1)],
                    kctx_sb[:, 128 * t : 128 * (t + 1)],
                    start=(t == 0),
                    stop=(t == 3),
                )
            ctx_sb = epool.tile([128, D], F32, tag="ctxs", name="ctx_sb")
            nc.vector.tensor_scalar_mul(ctx_sb[:], ctxp[:], rr[:])
            nc.sync.dma_start(out[128 * beta : 128 * (beta + 1), :], ctx_sb[:])

        # Bank A's exp fits the ACT gap after chunk 4, but its PE work
        # (transposes + context matmuls) is deferred until after the final
        # chunks so the terminal logits matmuls (expB's dependency) are not
        # queued behind it on the TensorEngine.
        for ci, n in enumerate(CHUNKS):
            emit_chunk(n, bias_mode=(ci == 0))
            if ci == 4:
                emit_tail_exp(0)
        emit_tail_exp(1)
        emit_tail_rest(0)
        emit_tail_rest(1)

    nc.compile()
    return nc


def _get_nc():
    global _CACHED_NC
    if _CACHED_NC is None:
        _CACHED_NC = _build_nc()
    return _CACHED_NC


def _build_w2d(w2):
    """(128, 256): slice g8 has column 4*g8+j = w2 on partitions [32j, 32j+32)."""
    w2d = np.zeros((128, 8 * H), np.float32)
    for g8 in range(8):
        for j in range(4):
            w2d[32 * j : 32 * j + 32, 32 * g8 + 4 * g8 + j] = w2
    return w2d


def _qmap():
    """out row r -> local query index."""
    r = np.arange(NQC)
    beta = r // 128
    p = r % 128
    return 64 * (p % 4) + 32 * beta + 8 * (p // 32) + (p % 32) // 4


def _in_maps(keys, queries, Wk, Wq, b1, w2):
    keys = np.asarray(keys, np.float32)
    queries = np.asarray(queries, np.float32)
    Wk = np.asarray(Wk, np.float32)
    Wq = np.asarray(Wq, np.float32)
    b1 = np.asarray(b1, np.float32)
    w2 = np.asarray(w2, np.float32)

    bundleB = np.zeros((128, NCOLB), np.float32)
    bundleB[:, W2D0 : W2D0 + 8 * H] = _build_w2d(w2)
    bundleB[:, ID0 : ID0 + 128] = np.eye(128, dtype=np.float32)
    b14 = np.tile(b1, 4)  # (128,)

    maps = []
    for c in range(8):
        b, half = divmod(c, 2)
        kb = keys[b]  # (512, 128)
        bA = np.zeros((128, NCOLA), np.float32)
        bA[:, KT0 : KT0 + NK] = kb.T
        bA[:, QT0 : QT0 + NQC] = queries[b, NQC * half : NQC * (half + 1)].T
        bA[:, WK0 : WK0 + H] = Wk
        bA[:, WQ0 : WQ0 + H] = Wq
        bA[:, B140] = b14
        bB = bundleB.copy()
        bB[:, KX0 : KX0 + NK] = (
            kb.reshape(4, 128, 128).transpose(1, 0, 2).reshape(128, 512)
        )
        maps.append(
            {
                "bundleA": bA.astype(ml_dtypes.bfloat16),
                "bundleB": bB.astype(ml_dtypes.bfloat16),
            }
        )
    return maps


def _run(in_maps, trace=False):
    nc = _get_nc()
    return run_bass_kernel_spmd(nc, in_maps, core_ids=list(range(8)), trace=trace)


def kernel(keys, queries, Wk, Wq, b1, w2, b2):
    res = _run(_in_maps(keys, queries, Wk, Wq, b1, w2))
    qmap = _qmap()
    outv = np.empty((B, NQ, D), np.float32)
    for c in range(8):
        b, half = divmod(c, 2)
        outv[b, NQC * half + qmap] = res.results[c]["out"]
    return outv



# revision 9
# speedup vs baseline: 2.1202x; 2.1202x over previous
"""Additive-attention (ContentAttender) Bass kernel for 8 TRN2 NeuronCores.

Problem: B=4, NQ=512, NK=512, D=128, H=32
  logits[b,q,k] = w2 . tanh(qh[b,q] + kh[b,k] + b1) + b2
  out = softmax_k(logits) @ keys

Sharding: data-parallel over (batch x query-half) -> 8 cores, each core
handles one batch's 256 queries vs all 512 keys. No collectives.

Method: the O(NQ*NK*H) tanh tensor is never materialized. Since the score
is a function of a SUM (qh + kh), expand tanh in a separable trig basis:
  tanh(s) ~= sum_m c_m sin(w_m s),  sin(w(a+b)) = sin(wa)cos(wb)+cos(wa)sin(wb)
(M=4 free-fitted frequencies, end-to-end rel err ~2.5e-3, at the bf16
floor). Each core then only evaluates sin/cos features on the small
qh [256,32] / kh [512,32] tensors and contracts the 2M*H=256 feature dim
on the TensorEngine. Phases are built by PE matmuls with omega-prescaled
replicated weights (fp32 PSUM), range-reduced into [-pi,pi] by single
DVE add_range_wrap ops (HW Sin is only accurate to ~|3.3|), activated by
ACT Sin (b1 folded into the per-partition bias). Logits come out
TRANSPOSED (k on partitions), so softmax rowsums and the context matmuls
need no transposes: ones-matmul colsums give [q,1] directly and the
context is st=E^T chunks @ keys. Output rows are in natural query order.
"""

import contextlib

import numpy as np
import ml_dtypes

import concourse.bass as bass  # noqa: F401
import concourse.mybir as mybir
import concourse.tile as tile
from concourse import bacc
from concourse.bass_utils import run_bass_kernel_spmd

F32 = mybir.dt.float32
BF16 = mybir.dt.bfloat16
AF = mybir.ActivationFunctionType

B, NQ, NK, D, H = 4, 512, 512, 128, 32
NQC = NQ // 2          # queries per core = 256
M = 4                  # trig terms; feature dim = 2*M*H = 256

# fitted tanh(s) ~= sum_m OMEGA_C[1,m] * sin(OMEGA_C[0,m] * s)
OMEGA = np.array([0.58658092, 0.58659907, 1.83957819, 3.31890976])
COEF = np.array([1.15549101, -0.0317051, 0.14888519, 0.01860145])

PI = float(np.pi)

_CACHED_NC = None


def _build_nc():
    nc = bacc.Bacc("TRN2", target_bir_lowering=False, debug=False)

    wmat = nc.declare_dram_parameter("wmat", [128, 2 * D + 4], BF16, isOutput=False)
    kTp = nc.declare_dram_parameter("kT", [128, NK], BF16, isOutput=False)
    qTp = nc.declare_dram_parameter("qT", [128, NQC], BF16, isOutput=False)
    kctxp = nc.declare_dram_parameter("kctx", [128, NK], BF16, isOutput=False)
    vecsp = nc.declare_dram_parameter("vecs", [128, 2], F32, isOutput=False)
    out = nc.declare_dram_parameter("out", [NQC, D], F32, isOutput=True)

    with tile.TileContext(nc) as tc, contextlib.ExitStack() as ctx:
        cpool = ctx.enter_context(tc.tile_pool(name="consts", bufs=1))
        wpool = ctx.enter_context(tc.tile_pool(name="wraps", bufs=1))
        fpool = ctx.enter_context(tc.tile_pool(name="feats", bufs=1))
        epool = ctx.enter_context(tc.tile_pool(name="softmax", bufs=1))
        ps_b = ctx.enter_context(tc.tile_pool(name="ps_b", bufs=1, space="PSUM"))
        ps_a = ctx.enter_context(tc.tile_pool(name="ps_a", bufs=1, space="PSUM"))
        ps_l = ctx.enter_context(tc.tile_pool(name="ps_l", bufs=1, space="PSUM"))
        ps_t = ctx.enter_context(tc.tile_pool(name="ps_t", bufs=1, space="PSUM"))

        wm = cpool.tile([128, 2 * D + 4], BF16, tag="wm")
        nc.sync.dma_start(wm[:], wmat[:])
        kT = cpool.tile([128, NK], BF16, tag="kT")
        nc.sync.dma_start(kT[:], kTp[:])
        qT = cpool.tile([128, NQC], BF16, tag="qT")
        nc.sync.dma_start(qT[:], qTp[:])
        vecs = cpool.tile([128, 2], F32, tag="vecs")
        nc.sync.dma_start(vecs[:], vecsp[:])
        kctx = cpool.tile([128, NK], BF16, tag="kctx")
        nc.sync.dma_start(kctx[:], kctxp[:])

        WkO = wm[:, 0:D]
        WqO = wm[:, D : 2 * D]
        ones = wm[:, 2 * D : 2 * D + 1]
        cw = vecs[:, 0:1]
        biasA = vecs[:, 1:2]

        # phases: PB[(m,h), k] = omega_m*kh[k,h]; PA[(m,h), q] = omega_m*qh[q,h]
        PB = ps_b.tile([128, NK], F32, tag="PB", name="PB")
        nc.tensor.matmul(PB[:], WkO, kT, start=True, stop=True)
        PA = ps_a.tile([128, NQC], F32, tag="PA", name="PA")
        nc.tensor.matmul(PA[:], WqO, qT, start=True, stop=True)

        # range-reduce into [-pi, pi]; cos-tiles get +pi/2 shift pre-wrap
        WBS = wpool.tile([128, NK], F32, tag="WBS")
        nc.vector.add_range_wrap(WBS[:], PB[:], 0.0, PI, 2 * PI)
        WAS = wpool.tile([128, NQC], F32, tag="WAS")
        nc.vector.add_range_wrap(WAS[:], PA[:], 0.0, PI, 2 * PI)
        WAC = wpool.tile([128, NQC], F32, tag="WAC")
        nc.vector.add_range_wrap(WAC[:], PA[:], PI / 2, PI, 2 * PI)
        WBC = wpool.tile([128, NK], F32, tag="WBC")
        nc.vector.add_range_wrap(WBC[:], PB[:], PI / 2, PI, 2 * PI)

        # features (bf16); a-side adds omega_m*b1[h] in the ACT bias
        BS = fpool.tile([128, NK], BF16, tag="BS")
        nc.scalar.activation(BS[:], WBS[:], AF.Sin)
        AS = fpool.tile([128, NQC], BF16, tag="AS")
        nc.scalar.activation(AS[:], WAS[:], AF.Sin, bias=biasA)
        AC = fpool.tile([128, NQC], BF16, tag="AC")
        nc.scalar.activation(AC[:], WAC[:], AF.Sin, bias=biasA)
        BC = fpool.tile([128, NK], BF16, tag="BC")
        nc.scalar.activation(BC[:], WBC[:], AF.Sin)

        # fold c_m*w2[h] into the a-side features
        ACm = fpool.tile([128, NQC], BF16, tag="ACm")
        nc.vector.tensor_scalar_mul(ACm[:], AC[:], cw)
        ASm = fpool.tile([128, NQC], BF16, tag="ASm")
        nc.vector.tensor_scalar_mul(ASm[:], AS[:], cw)

        # logits^T[k, q] = sum_f Bfeat[f,k]*Afeat[f,q], 4 k-chunks packed
        # two per PSUM bank
        LA = ps_l.tile([128, 2 * NQC], F32, tag="LA", name="LA")
        LB = ps_l.tile([128, 2 * NQC], F32, tag="LB", name="LB")
        L = [
            LA[:, 0:NQC], LA[:, NQC : 2 * NQC],
            LB[:, 0:NQC], LB[:, NQC : 2 * NQC],
        ]
        for kc in range(4):
            nc.tensor.matmul(
                L[kc], BS[:, 128 * kc : 128 * (kc + 1)], ACm[:],
                start=True, stop=False,
            )
            nc.tensor.matmul(
                L[kc], BC[:, 128 * kc : 128 * (kc + 1)], ASm[:],
                start=False, stop=True,
            )

        # exp (no max-subtraction: |logits| <= ~3.2)
        E = []
        for kc in range(4):
            Ek = epool.tile([128, NQC], BF16, tag=f"E{kc}", name=f"E{kc}")
            nc.scalar.activation(Ek[:], L[kc], AF.Exp)
            E.append(Ek)

        # rowsums rs[q,1] = sum_k E^T[k,q] via ones-matmuls; context per q-half
        T = ps_t.tile([128, 2 * D + 2], F32, tag="T", name="T")
        for qh_ in range(2):
            ql = slice(128 * qh_, 128 * (qh_ + 1))
            rsp = T[:, 2 * D + qh_ : 2 * D + qh_ + 1]
            for kc in range(4):
                nc.tensor.matmul(
                    rsp, E[kc][:, ql], ones, start=(kc == 0), stop=(kc == 3)
                )
            ctxp = T[:, D * qh_ : D * (qh_ + 1)]
            for kc in range(4):
                nc.tensor.matmul(
                    ctxp, E[kc][:, ql], kctx[:, 128 * kc : 128 * (kc + 1)],
                    start=(kc == 0), stop=(kc == 3),
                )
            rr = epool.tile([128, 1], F32, tag=f"rr{qh_}", name=f"rr{qh_}")
            nc.vector.reciprocal(rr[:], rsp)
            ctxs = epool.tile([128, D], F32, tag=f"ctxs{qh_}", name=f"ctxs{qh_}")
            nc.vector.tensor_scalar_mul(ctxs[:], ctxp, rr[:])
            nc.sync.dma_start(out[128 * qh_ : 128 * (qh_ + 1), :], ctxs[:])

    nc.compile()
    return nc


def _get_nc():
    global _CACHED_NC
    if _CACHED_NC is None:
        _CACHED_NC = _build_nc()
    return _CACHED_NC


def _in_maps(keys, queries, Wk, Wq, b1, w2):
    keys = np.asarray(keys, np.float32)
    queries = np.asarray(queries, np.float32)
    Wk = np.asarray(Wk, np.float32)
    Wq = np.asarray(Wq, np.float32)
    b1 = np.asarray(b1, np.float32)
    w2 = np.asarray(w2, np.float32)

    # wmat: [WkO | WqO | ones | pad]; partition 32m+h carries omega_m scale
    om_part = np.repeat(OMEGA, H).astype(np.float32)           # (128,)
    cw_part = np.repeat(COEF, H).astype(np.float32) * np.tile(w2, M)
    bias_part = om_part * np.tile(b1, M)

    wmat = np.zeros((128, 2 * D + 4), np.float32)
    # WkO[d, 32m+h] = omega_m * Wk[d, h]
    WkO = np.concatenate([o * Wk for o in OMEGA], axis=1)      # (D, 128)
    WqO = np.concatenate([o * Wq for o in OMEGA], axis=1)      # (D, 128)
    wmat[:, 0:D] = WkO
    wmat[:, D : 2 * D] = WqO
    wmat[:, 2 * D] = 1.0

    vecs = np.zeros((128, 2), np.float32)
    vecs[:, 0] = cw_part
    vecs[:, 1] = bias_part

    maps = []
    for c in range(8):
        b, half = divmod(c, 2)
        kb = keys[b]  # (512, 128)
        maps.append(
            {
                "wmat": wmat.astype(ml_dtypes.bfloat16),
                "kT": kb.T.astype(ml_dtypes.bfloat16),
                "qT": queries[b, NQC * half : NQC * (half + 1)].T.astype(
                    ml_dtypes.bfloat16
                ),
                "kctx": kb.reshape(4, 128, 128)
                .transpose(1, 0, 2)
                .reshape(128, 512)
                .astype(ml_dtypes.bfloat16),
                "vecs": vecs,
            }
        )
    return maps


def _run(in_maps, trace=False):
    nc = _get_nc()
    return run_bass_kernel_spmd(nc, in_maps, core_ids=list(range(8)), trace=trace)


def kernel(keys, queries, Wk, Wq, b1, w2, b2):
    res = _run(_in_maps(keys, queries, Wk, Wq, b1, w2))
    outv = np.empty((B, NQ, D), np.float32)
    for c in range(8):
        b, half = divmod(c, 2)
        outv[b, NQC * half : NQC * (half + 1)] = res.results[c]["out"]
    return outv


# revision 12
# speedup vs baseline: 2.2167x; 1.0455x over previous
"""Additive-attention (ContentAttender) Bass kernel for 8 TRN2 NeuronCores.

Problem: B=4, NQ=512, NK=512, D=128, H=32
  logits[b,q,k] = w2 . tanh(qh[b,q] + kh[b,k] + b1) + b2
  out = softmax_k(logits) @ keys

Sharding: data-parallel over (batch x query-half) -> 8 cores, each core
handles one batch's 256 queries vs all 512 keys. No collectives.

Method: the O(NQ*NK*H) tanh tensor is never materialized. Since the score
is a function of a SUM (qh + kh), expand tanh in a separable trig basis:
  tanh(s) ~= sum_m c_m sin(w_m s),  sin(w(a+b)) = sin(wa)cos(wb)+cos(wa)sin(wb)
(M=4 free-fitted frequencies, end-to-end rel err ~2.4e-3, at the bf16
floor). Each core then only evaluates sin/cos features on the small
qh [256,32] / kh [512,32] tensors and contracts the 2M*H=256 feature dim
on the TensorEngine. Phases are built by PE matmuls with omega-prescaled
replicated weights (fp32 PSUM), range-reduced into [-pi,pi] by single
DVE add_range_wrap ops (HW Sin is only accurate to ~|3.3|; cos-tiles get
the +pi/2 as the wrap shift), activated by ACT Sin (b1 folded into the
per-partition bias). Logits come out TRANSPOSED (k on partitions), so
softmax needs no transposes: the rowsum is a ones-column appended to the
keys in the context matmul moving operand. Input DMAs are issued from
five different engine queues so their ~0.6us issue slots overlap; the
Sin->Exp ACT table switch (~1.3us, unavoidable: no table set holds both)
overlaps the logits matmuls.
"""

import contextlib

import numpy as np
import ml_dtypes

import concourse.bass as bass  # noqa: F401
import concourse.mybir as mybir
import concourse.tile as tile
from concourse import bacc
from concourse.bass_utils import run_bass_kernel_spmd

F32 = mybir.dt.float32
BF16 = mybir.dt.bfloat16
AF = mybir.ActivationFunctionType

B, NQ, NK, D, H = 4, 512, 512, 128, 32
NQC = NQ // 2          # queries per core = 256
M = 4                  # trig terms; feature dim = 2*M*H = 256

# fitted tanh(s) ~= sum_m COEF[m] * sin(OMEGA[m] * s)
OMEGA = np.array([0.58658092, 0.58659907, 1.83957819, 3.31890976])
COEF = np.array([1.15549101, -0.0317051, 0.14888519, 0.01860145])

PI = float(np.pi)

_CACHED_NC = None


def _build_nc():
    nc = bacc.Bacc("TRN2", target_bir_lowering=False, debug=False)

    wmat = nc.declare_dram_parameter("wmat", [128, 2 * D], BF16, isOutput=False)
    kTp = nc.declare_dram_parameter("kT", [128, NK], BF16, isOutput=False)
    qTp = nc.declare_dram_parameter("qT", [128, NQC], BF16, isOutput=False)
    kctxp = nc.declare_dram_parameter("kctx", [128, 4 * 129], BF16, isOutput=False)
    vecsp = nc.declare_dram_parameter("vecs", [128, 2], F32, isOutput=False)
    out = nc.declare_dram_parameter("out", [NQC, D], F32, isOutput=True)

    with tile.TileContext(nc) as tc, contextlib.ExitStack() as ctx:
        cpool = ctx.enter_context(tc.tile_pool(name="consts", bufs=1))
        wpool = ctx.enter_context(tc.tile_pool(name="wraps", bufs=1))
        fpool = ctx.enter_context(tc.tile_pool(name="feats", bufs=1))
        epool = ctx.enter_context(tc.tile_pool(name="softmax", bufs=1))
        ps_b = ctx.enter_context(tc.tile_pool(name="ps_b", bufs=1, space="PSUM"))
        ps_a = ctx.enter_context(tc.tile_pool(name="ps_a", bufs=1, space="PSUM"))
        ps_l = ctx.enter_context(tc.tile_pool(name="ps_l", bufs=1, space="PSUM"))
        ps_t = ctx.enter_context(tc.tile_pool(name="ps_t", bufs=1, space="PSUM"))

        # input DMAs issued from three different queues so they overlap
        # (only sync/scalar/gpsimd may issue DMAs)
        kT = cpool.tile([128, NK], BF16, tag="kT")
        nc.sync.dma_start(kT[:], kTp[:])
        wm = cpool.tile([128, 2 * D], BF16, tag="wm")
        nc.scalar.dma_start(wm[:], wmat[:])
        qT = cpool.tile([128, NQC], BF16, tag="qT")
        nc.gpsimd.dma_start(qT[:], qTp[:])
        vecs = cpool.tile([128, 2], F32, tag="vecs")
        nc.gpsimd.dma_start(vecs[:], vecsp[:])
        kctx = cpool.tile([128, 4 * 129], BF16, tag="kctx")
        nc.gpsimd.dma_start(kctx[:], kctxp[:])

        WkO = wm[:, 0:D]
        WqO = wm[:, D : 2 * D]
        cw = vecs[:, 0:1]
        biasA = vecs[:, 1:2]

        # phases: PB[(m,h), k] = omega_m*kh[k,h]; PA[(m,h), q] = omega_m*qh[q,h]
        PB = ps_b.tile([128, NK], F32, tag="PB", name="PB")
        nc.tensor.matmul(PB[:], WkO, kT, start=True, stop=True)
        PA = ps_a.tile([128, NQC], F32, tag="PA", name="PA")
        nc.tensor.matmul(PA[:], WqO, qT, start=True, stop=True)

        # range-reduce into [-pi, pi]; cos-tiles get +pi/2 shift pre-wrap
        WBS = wpool.tile([128, NK], F32, tag="WBS")
        nc.vector.add_range_wrap(WBS[:], PB[:], 0.0, PI, 2 * PI)
        WA = wpool.tile([128, 2 * NQC], F32, tag="WA")
        nc.vector.add_range_wrap(WA[:, 0:NQC], PA[:], 0.0, PI, 2 * PI)
        nc.vector.add_range_wrap(WA[:, NQC : 2 * NQC], PA[:], PI / 2, PI, 2 * PI)
        WBC = wpool.tile([128, NK], F32, tag="WBC")
        nc.vector.add_range_wrap(WBC[:], PB[:], PI / 2, PI, 2 * PI)

        # features (bf16); a-side adds omega_m*b1[h] via the ACT bias and
        # folds c_m*w2[h] with one DVE multiply: Am = [ASm | ACm]
        BS = fpool.tile([128, NK], BF16, tag="BS")
        nc.scalar.activation(BS[:], WBS[:], AF.Sin)
        A = fpool.tile([128, 2 * NQC], BF16, tag="A")
        nc.scalar.activation(A[:], WA[:], AF.Sin, bias=biasA)
        BC = fpool.tile([128, NK], BF16, tag="BC")
        nc.scalar.activation(BC[:], WBC[:], AF.Sin)
        Am = fpool.tile([128, 2 * NQC], BF16, tag="Am")
        nc.vector.tensor_scalar_mul(Am[:], A[:], cw)
        ASm = Am[:, 0:NQC]
        ACm = Am[:, NQC : 2 * NQC]

        # logits^T[k, q] = sum_f Bfeat[f,k]*Afeat[f,q], 4 k-chunks packed
        # two per PSUM bank
        LA = ps_l.tile([128, 2 * NQC], F32, tag="LA", name="LA")
        LB = ps_l.tile([128, 2 * NQC], F32, tag="LB", name="LB")
        L = [
            LA[:, 0:NQC], LA[:, NQC : 2 * NQC],
            LB[:, 0:NQC], LB[:, NQC : 2 * NQC],
        ]
        for kc in range(4):
            nc.tensor.matmul(
                L[kc], BS[:, 128 * kc : 128 * (kc + 1)], ACm,
                start=True, stop=False,
            )
            nc.tensor.matmul(
                L[kc], BC[:, 128 * kc : 128 * (kc + 1)], ASm,
                start=False, stop=True,
            )

        # exp (no max-subtraction: |logits| <= ~3.2); 2 wide instructions
        E01 = epool.tile([128, 2 * NQC], BF16, tag="E01", name="E01")
        nc.scalar.activation(E01[:], LA[:], AF.Exp)
        E23 = epool.tile([128, 2 * NQC], BF16, tag="E23", name="E23")
        nc.scalar.activation(E23[:], LB[:], AF.Exp)

        def e_chunk(kc, qh_):
            t = E01 if kc < 2 else E23
            c0 = NQC * (kc % 2) + 128 * qh_
            return t[:, c0 : c0 + 128]

        # fused context+rowsum: kctx chunk kc = [keys_chunk | ones], so
        # T[qh][:, 0:128] = context, col 128 = softmax denominator.
        # One PSUM bank per q-half: interleaved accumulation chains must
        # not share a bank.
        T = [
            ps_t.tile([128, 129], F32, tag=f"T{qh_}", name=f"T{qh_}")
            for qh_ in range(2)
        ]
        for kc in range(4):
            for qh_ in range(2):
                nc.tensor.matmul(
                    T[qh_][:],
                    e_chunk(kc, qh_),
                    kctx[:, 129 * kc : 129 * (kc + 1)],
                    start=(kc == 0), stop=(kc == 3),
                )
        for qh_ in range(2):
            rr = epool.tile([128, 1], F32, tag=f"rr{qh_}", name=f"rr{qh_}")
            nc.vector.reciprocal(rr[:], T[qh_][:, 128:129])
            ctxs = epool.tile([128, D], F32, tag=f"ctxs{qh_}", name=f"ctxs{qh_}")
            nc.vector.tensor_scalar_mul(ctxs[:], T[qh_][:, 0:128], rr[:])
            eng = nc.sync if qh_ == 0 else nc.scalar
            eng.dma_start(out[128 * qh_ : 128 * (qh_ + 1), :], ctxs[:])

    nc.compile()
    return nc


def _get_nc():
    global _CACHED_NC
    if _CACHED_NC is None:
        _CACHED_NC = _build_nc()
    return _CACHED_NC


def _in_maps(keys, queries, Wk, Wq, b1, w2):
    keys = np.asarray(keys, np.float32)
    queries = np.asarray(queries, np.float32)
    Wk = np.asarray(Wk, np.float32)
    Wq = np.asarray(Wq, np.float32)
    b1 = np.asarray(b1, np.float32)
    w2 = np.asarray(w2, np.float32)

    om_part = np.repeat(OMEGA, H).astype(np.float32)           # (128,)
    cw_part = np.repeat(COEF, H).astype(np.float32) * np.tile(w2, M)
    bias_part = om_part * np.tile(b1, M)

    # wmat: [WkO | WqO]; WkO[d, 32m+h] = omega_m * Wk[d, h]
    wmat = np.zeros((128, 2 * D), np.float32)
    wmat[:, 0:D] = np.concatenate([o * Wk for o in OMEGA], axis=1)
    wmat[:, D : 2 * D] = np.concatenate([o * Wq for o in OMEGA], axis=1)

    vecs = np.zeros((128, 2), np.float32)
    vecs[:, 0] = cw_part
    vecs[:, 1] = bias_part

    maps = []
    for c in range(8):
        b, half = divmod(c, 2)
        kb = keys[b]  # (512, 128)
        kctx = np.ones((128, 4, 129), np.float32)
        kctx[:, :, :128] = kb.reshape(4, 128, 128).transpose(1, 0, 2)
        maps.append(
            {
                "wmat": wmat.astype(ml_dtypes.bfloat16),
                "kT": kb.T.astype(ml_dtypes.bfloat16),
                "qT": queries[b, NQC * half : NQC * (half + 1)].T.astype(
                    ml_dtypes.bfloat16
                ),
                "kctx": kctx.reshape(128, 4 * 129).astype(ml_dtypes.bfloat16),
                "vecs": vecs,
            }
        )
    return maps


def _run(in_maps, trace=False):
    nc = _get_nc()
    return run_bass_kernel_spmd(nc, in_maps, core_ids=list(range(8)), trace=trace)


def kernel(keys, queries, Wk, Wq, b1, w2, b2):
    res = _run(_in_maps(keys, queries, Wk, Wq, b1, w2))
    outv = np.empty((B, NQ, D), np.float32)
    for c in range(8):
        b, half = divmod(c, 2)
        outv[b, NQC * half : NQC * (half + 1)] = res.results[c]["out"]
    return outv
